# revision 48
# baseline (speedup 1.0000x reference)
"""Trainium2 Bass kernel for nn_ClothGraphConvNetwork_MLPDecoder.

8 NeuronCores, data-parallel over batch (2 batches/core), no collectives.

v2 design (on top of v1's separable-lin0 / dense-adjacency structure):
- Lockstep batch-pair interleave: the two per-core batches are emitted
  phase-alternately so one batch's matmuls hide the other's GroupNorm
  chain latency (v1 lost ~390us/rep to PE idle gaps).
- Adjacency matmul: asb stored fp8e4m3 (adj_w = 1/8 is exact, halves
  SBUF), sup bf16, kt-outer loop so stationary sup tiles load 4x less
  often. Full fp8+DoubleRow was tried and reverted: sup quantization
  noise amplifies ~1.4x per block through the GroupNorms and blew the
  2e-2 budget.
- b0 skip path from host-precomputed skW@W0v / skW@W0img: SU =
  (skW@W0v)@verts once per rep; the per-batch image part collapses into
  a per-channel bias (svbh) folded into the b0 lin2 evacuation. v_b is
  host-computed in f32 (vbh): the b0 GroupNorm divides channel-mean
  differences by a tiny vertex-sigma (~0.035), amplifying any v_b
  rounding ~30x, so it must not pass through bf16.
- dtypes: trunk x / su-residual path / y2 / lin2 weights / head in f32r
  (errors there feed the residual trunk and re-amplify); conv-path
  y1 / xr / lin1-conv weights in bf16 (their noise is attenuated ~4x
  re-entering the trunk).
- Shortened GN chains: PSUM-direct reads, scalar_tensor_tensor fusions,
  negated-mean trick (beta = (-m)*a + b in one op).
- Engine tables route relu-affine applies across ACT/DVE; GPSIMD proved
  too slow (0.42 efficiency + 95ns launch) for anything latency-gating.
- Head split in two stages so its GN chain hides under the other
  batch's lin2; residual identity matmuls run two chunks ahead.
"""

import contextlib

import numpy as np
import ml_dtypes

import concourse.bass as bass
import concourse.tile as tile
from concourse import bacc, mybir
from concourse.bass_utils import run_bass_kernel_spmd

F32R = mybir.dt.float32r
F32 = mybir.dt.float32
BF16 = mybir.dt.bfloat16
FP8 = mybir.dt.float8e4
AF = mybir.ActivationFunctionType
ALU = mybir.AluOpType
DR = mybir.MatmulPerfMode.DoubleRow

B, N, DEG = 16, 1723, 8
C, L, H = 512, 5, 256
NP = 1724              # padded vertex count
NCORES = 8
BLOC = B // NCORES     # batches per core
NT = 14                # vertex 128-tiles (last has 59 real rows)
NPAIR = 7              # DoubleRow k-tile pairs
FCH = [(0, 432), (432, 432), (864, 432), (1296, 428)]


def _param_layout():
    items = [("lin0_b", 1024),
             ("b0_pre_g", 1024), ("b0_pre_b", 1024),
             ("b0_lin1_b", 256), ("b0_n1_g", 256), ("b0_n1_b", 256),
             ("b0_conv_b", 256), ("b0_n2_g", 256), ("b0_n2_b", 256),
             ("b0_sklin2_b", 512)]
    for i in range(L):
        items += [(f"blk_pre_g{i}", 512), (f"blk_pre_b{i}", 512),
                  (f"blk_lin1_b{i}", 256), (f"blk_n1_g{i}", 256),
                  (f"blk_n1_b{i}", 256), (f"blk_conv_b{i}", 256),
                  (f"blk_n2_g{i}", 256), (f"blk_n2_b{i}", 256),
                  (f"blk_lin2_b{i}", 512)]
    items += [("h1_b", 64), ("h2_b", 32), ("hn_g", 32), ("hn_b", 32),
              ("h3_b", 3)]
    idx = {}
    pos = 0
    for name, ln in items:
        for t in range((ln + 127) // 128):
            idx[(name, t)] = pos
            pos += 1
    return items, idx, pos


PARAM_ITEMS, PIDX, NSLOT = _param_layout()
PHASES = []
FUSE_MT = False

# engine tables for relu-affine applies: "a"=ACT, "v"=DVE, "g"=GPSIMD
ENG_XR = {0: ("v", "a", "v", "a"), 1: ("a", "v", "a", "v")}   # [b][ct]
ENG_GN2 = {0: ("v", "a"), 1: ("a", "v")}                      # [ct][half]
ENG_GN3 = {0: ("v", "a"), 1: ("a", "v")}                      # [dt][half]
ENG_X0R = ("a", "v")                                          # [b]
HALVES = [(0, 864), (864, 860)]                               # FCH-aligned


def build(nreps=1, fp8agg=True, xf32=True, dump=0):
    nc = bacc.Bacc("TRN2", target_bir_lowering=False, debug=False)
    PHASES.clear()
    AGG_DT = FP8 if fp8agg else BF16
    X_DT = F32R if xf32 else BF16

    def _mark(label):
        PHASES.append((label, nc.next_id()))

    d = {}

    def din(name, shape, dt):
        d[name] = nc.dram_tensor(name, list(shape), dt, kind="ExternalInput")

    din("verts", (4, NP), F32R)
    din("at", (NT, 128, NP), AGG_DT)
    din("w0vt", (4, 1024), F32R)
    din("swt", (4, 512), F32R)
    din("vbh", (128, 8, BLOC), F32)
    din("svbh", (128, 4, BLOC), F32)
    din("g8", (128, 16), F32)     # indicator / 8  (group-mean reduce)
    din("g8t", (16, 128), F32)    # 0/1 indicator transpose (broadcast)
    din("identb", (128, 128), BF16)
    if xf32:
        din("identr", (128, 128), F32R)
    din("prm", (128, NSLOT), F32)
    din("b0l1t", (8, 128, H), BF16)
    din("b0cw", (2, 128, H), BF16)
    din("b0l2t", (2, 128, C), F32R)
    din("bl1t", (L, 4, 128, H), BF16)
    din("bcw", (L, 2, 128, H), BF16)
    din("bl2t", (L, 2, 128, C), F32R)
    din("h1t", (4, 128, 64), X_DT)
    din("h2t", (64, 32), F32R)
    din("h3t", (32, 4), F32R)
    out_d = nc.dram_tensor("out", [BLOC, 3, N], F32, kind="ExternalOutput")
    dbg_d = None
    if dump:
        dbg_d = nc.dram_tensor("dbg", [16, 128, NP], BF16,
                               kind="ExternalOutput")

    with tile.TileContext(nc) as tc, contextlib.ExitStack() as ctx:
        cons = ctx.enter_context(tc.tile_pool(name="cons", bufs=1))
        ps = ctx.enter_context(tc.tile_pool(name="ps", bufs=6, space="PSUM"))
        psc = ctx.enter_context(tc.tile_pool(name="psc", bufs=2, space="PSUM"))
        sm = ctx.enter_context(tc.tile_pool(name="sm", bufs=2))
        xp = ctx.enter_context(tc.tile_pool(name="xp", bufs=8))
        yp = ctx.enter_context(tc.tile_pool(name="yp", bufs=4))
        supp = ctx.enter_context(tc.tile_pool(name="supp", bufs=2))
        xrp = ctx.enter_context(tc.tile_pool(name="xrp", bufs=8))
        wp = ctx.enter_context(tc.tile_pool(name="wp", bufs=2))

        EV = {"v": nc.vector, "g": nc.gpsimd}

        def evac(e, dst, src, b_ap):
            if e == "a":
                nc.scalar.activation(dst, src, AF.Identity, bias=b_ap)
            else:
                nc.vector.tensor_scalar(dst, src, b_ap, None, op0=ALU.add)

        def apply_ra(e, dst, src, a_ap, b_ap):
            """dst = relu(a*src + b); a/b per-partition (p,1) APs."""
            if e == "a":
                nc.scalar.activation(dst, src, AF.Relu, bias=b_ap, scale=a_ap)
            else:
                EV[e].tensor_scalar(dst, src, a_ap, b_ap,
                                    op0=ALU.mult, op1=ALU.add)
                EV[e].tensor_scalar_max(dst, dst, 0.0)

        # ---- constants ----
        g8 = cons.tile([128, 16], F32)
        nc.sync.dma_start(g8[:], d["g8"].ap())
        g8t = cons.tile([16, 128], F32)
        nc.sync.dma_start(g8t[:], d["g8t"].ap())
        identb = cons.tile([128, 128], BF16)
        nc.sync.dma_start(identb[:], d["identb"].ap())
        identx = identb
        if xf32:
            identx = cons.tile([128, 128], F32R)
            nc.sync.dma_start(identx[:], d["identr"].ap())
        prm = cons.tile([128, NSLOT], F32)
        nc.sync.dma_start(prm[:], d["prm"].ap())
        verts = cons.tile([4, NP], F32R)
        nc.sync.dma_start(verts[:], d["verts"].ap())
        w0vt = cons.tile([4, 1024], F32R)
        nc.sync.dma_start(w0vt[:], d["w0vt"].ap())
        swt = cons.tile([4, 512], F32R)
        nc.sync.dma_start(swt[:], d["swt"].ap())
        asb = cons.tile([128, NT, NP], AGG_DT)
        for kt in range(NT):
            nc.sync.dma_start(asb[:, kt, :], d["at"].ap()[kt])
        b0l1 = cons.tile([128, 8, H], BF16)
        for kt in range(8):
            nc.sync.dma_start(b0l1[:, kt, :], d["b0l1t"].ap()[kt])
        cw0 = cons.tile([128, 2, H], BF16)
        for ct in range(2):
            nc.sync.dma_start(cw0[:, ct, :], d["b0cw"].ap()[ct])
        l2t0 = cons.tile([128, 2, C], F32R)
        for ct in range(2):
            nc.sync.dma_start(l2t0[:, ct, :], d["b0l2t"].ap()[ct])
        h1w = cons.tile([128, 4, 64], X_DT)
        for kt in range(4):
            nc.sync.dma_start(h1w[:, kt, :], d["h1t"].ap()[kt])
        h2w = cons.tile([64, 32], F32R)
        nc.sync.dma_start(h2w[:], d["h2t"].ap())
        h3w = cons.tile([32, 4], F32R)
        nc.sync.dma_start(h3w[:], d["h3t"].ap())
        eps = cons.tile([128, 1], F32)
        nc.vector.memset(eps[:], 1e-5)
        vb = cons.tile([128, 8, BLOC], F32, name="vb")
        nc.sync.dma_start(vb[:], d["vbh"].ap())
        svb2 = cons.tile([128, 4, BLOC], F32, name="svb2")
        nc.sync.dma_start(svb2[:], d["svbh"].ap())

        def P(name, t=0, parts=128, width=1):
            i = PIDX[(name, t)]
            return prm[0:parts, i:i + width]

        # ---- GN chain helpers ----
        def gn_chain8(st3, gname, bname, abtag):
            """Batched T=8 chain for b0gn1: st3 (128,8,2) = [mean, E2]."""
            G, T = 16, 8
            psg = psc.tile([16, 8, 2], F32, tag="psc", name="psg8")
            nc.tensor.matmul(psg[0:G, 0:T, :], g8[:, 0:G],
                             st3[:, 0:T, :], start=True, stop=True)
            pg = sm.tile([16, 8, 2], F32, tag="pg8", bufs=2, name="pg8")
            nc.vector.tensor_copy(pg[0:G, 0:T, :], psg[0:G, 0:T, :])
            t2 = sm.tile([16, 8], F32, tag="t28", bufs=2, name="t28")
            nc.vector.tensor_tensor(t2[0:G, 0:T], pg[0:G, 0:T, 0],
                                    pg[0:G, 0:T, 0], op=ALU.mult)
            nc.vector.tensor_tensor(t2[0:G, 0:T], pg[0:G, 0:T, 1],
                                    t2[0:G, 0:T], op=ALU.subtract)
            nc.scalar.activation(t2[0:G, 0:T], t2[0:G, 0:T], AF.Sqrt,
                                 bias=eps[0:G, :])
            mr = sm.tile([16, 8, 2], F32, tag="mr8", bufs=2, name="mr8")
            nc.vector.tensor_copy(mr[0:G, 0:T, 0], pg[0:G, 0:T, 0])
            nc.vector.reciprocal(mr[0:G, 0:T, 1], t2[0:G, 0:T])
            psb = psc.tile([128, 8, 2], F32, tag="psc", name="psb8")
            nc.tensor.matmul(psb[:, 0:T, :], g8t[0:G, :],
                             mr[0:G, 0:T, :], start=True, stop=True)
            ab = sm.tile([128, 8, 2], F32, tag=abtag, bufs=2, name="ab8")
            nc.vector.tensor_tensor(ab[:, 0:T, 0], psb[:, 0:T, 1],
                                    P(gname, 0, 128, T), op=ALU.mult)
            t3 = sm.tile([128, 8], F32, tag="t38", bufs=2, name="t38")
            nc.vector.tensor_tensor(t3[:, 0:T], psb[:, 0:T, 0],
                                    ab[:, 0:T, 0], op=ALU.mult)
            nc.vector.tensor_tensor(ab[:, 0:T, 1], P(bname, 0, 128, T),
                                    t3[:, 0:T], op=ALU.subtract)
            return ab

        def stats_new(T):
            return [sm.tile([128, 4, 6], F32, tag="stats", bufs=14,
                            name="sts") for _ in range(T)]

        def note(stt, ci, x_ap, f0, fw, parts=128):
            rw = fw if f0 + fw <= N else (N - f0)
            nc.vector.bn_stats(stt[0:parts, ci, :],
                               x_ap[0:parts, f0:f0 + rw])

        def gn_f1a(stt, parts=128, G=16):
            """aggregate -> group mean/rstd; returns mr (G,2) = [-m_g, rs_g]."""
            st = sm.tile([128, 1, 2], F32, tag="st1", bufs=8, name="st1")
            nc.vector.bn_aggr(st[0:parts, 0, :], stt[0:parts, :, :])
            # E2 = m*m + v
            nc.vector.scalar_tensor_tensor(
                st[0:parts, 0, 1:2], st[0:parts, 0, 0:1],
                st[0:parts, 0, 0:1], st[0:parts, 0, 1:2],
                op0=ALU.mult, op1=ALU.add)
            psg = psc.tile([16, 2], F32, tag="psc", name="psg1")
            nc.tensor.matmul(psg[0:G, :], g8[0:parts, 0:G],
                             st[0:parts, 0, :], start=True, stop=True)
            t2 = sm.tile([16, 2], F32, tag="t21", bufs=8, name="t21")
            nc.vector.tensor_scalar(t2[0:G, 0:1], psg[0:G, 0:1],
                                    psg[0:G, 0:1], None, op0=ALU.mult)
            # (E2 + eps) - m^2
            nc.vector.scalar_tensor_tensor(
                t2[0:G, 1:2], psg[0:G, 1:2], 1e-5, t2[0:G, 0:1],
                op0=ALU.add, op1=ALU.subtract)
            nc.scalar.activation(t2[0:G, 1:2], t2[0:G, 1:2], AF.Sqrt)
            mr = sm.tile([16, 2], F32, tag="mr1", bufs=8, name="mr1")
            nc.vector.reciprocal(mr[0:G, 1:2], t2[0:G, 1:2])
            nc.vector.tensor_scalar(mr[0:G, 0:1], psg[0:G, 0:1], -1.0, None,
                                    op0=ALU.mult)
            return mr

        def gn_f1b(mr, gname, gt, bname, parts=128, G=16,
                   abtag="ab", abbufs=6):
            psb = psc.tile([128, 2], F32, tag="psc", name="psb1")
            nc.tensor.matmul(psb[0:parts, :], g8t[0:G, 0:parts], mr[0:G, :],
                             start=True, stop=True)
            ab = sm.tile([128, 2], F32, tag=abtag, bufs=abbufs, name="ab1")
            nc.vector.tensor_scalar(ab[0:parts, 0:1], psb[0:parts, 1:2],
                                    P(gname, gt, parts), None, op0=ALU.mult)
            # beta = (-m)*a + b
            nc.vector.scalar_tensor_tensor(
                ab[0:parts, 1:2], psb[0:parts, 0:1], ab[0:parts, 0:1],
                P(bname, gt, parts), op0=ALU.mult, op1=ALU.add)
            return ab

        # ================= phase bodies =================

        def lin1_pair(pn, x, xab, klist):
            """y1[b] = lin1(relu-affine(x[b])) for both batches, lockstep.
            Per ci, batch A's evac+stats are emitted before batch B's
            matmuls so A's gn2 chain deps complete while B runs on PE."""
            y1 = {b: [yp.tile([128, NP], BF16, tag="y1", name="y1")
                      for _ in range(2)] for b in (0, 1)}
            ystats = {b: stats_new(2) for b in (0, 1)}
            for ci, (f0, fw) in enumerate(FCH):
                y1ps = {b: [ps.tile([128, 512], F32, tag="ps", name="y1ps")
                            for _ in range(2)] for b in (0, 1)}
                xr = {}
                for b in (0, 1):
                    for ct in range(4):
                        xr[(b, ct)] = xrp.tile([128, 432], BF16, tag="xr",
                                               name="xr")
                        apply_ra(ENG_XR[b][ct], xr[(b, ct)][:, :fw],
                                 x[b][ct][:, f0:f0 + fw],
                                 xab[b][ct][:, 0:1], xab[b][ct][:, 1:2])
                for b in (0, 1):
                    for ct in range(4):
                        for mt in range(2):
                            nc.tensor.matmul(
                                y1ps[b][mt][:, :fw],
                                klist(ct)[:, mt * 128:(mt + 1) * 128],
                                xr[(b, ct)][:, :fw],
                                start=(ct == 0), stop=(ct == 3))
                    for mt in range(2):
                        evac("a",
                             y1[b][mt][:, f0:f0 + fw],
                             y1ps[b][mt][:, :fw], P(pn("lin1_b"), mt))
                        note(ystats[b][mt], ci, y1[b][mt], f0, fw)
            return y1, ystats

        def tail_pair(pn, cw, l2w, xsrc, xdst, l2bias, y1, ystats,
                      pn_next, idm=None, post=None):
            """gn2 -> sup -> agg -> gn3 -> lin2 (+residual) for both batches.

            xsrc(b, mt) -> (128, NP) AP read for residual; xdst[b][mt] tiles
            written by lin2 evac with bias l2bias(b, mt)."""
            # --- gn2 + sup ---
            _mark("gn2")
            # all four chains first: their tiny PE matmuls sit right after
            # both batches' lin1 waves, deps long since complete
            ab2 = {}
            for b in (0, 1):
                ab2[b] = [gn_f1b(gn_f1a(ystats[b][ct]),
                                 pn("n1_g"), ct, pn("n1_b"))
                          for ct in range(2)]
            _mark("sup")
            sup = {}

            def sup_pairs(b, plist):
                for np_ in plist:
                    sps = ps.tile([128, 512], F32, tag="ps", name="sps")
                    for half in range(2):
                        nt = np_ * 2 + half
                        ms = nt * 128
                        mw = min(ms + 128, NP) - ms
                        for ct in range(2):
                            nc.tensor.matmul(
                                sps[0:mw, half * H:half * H + H],
                                y1[b][ct][:, ms:ms + mw], cw(ct),
                                start=(ct == 0), stop=(ct == 1))
                    for half in range(2):
                        nt = np_ * 2 + half
                        ms = nt * 128
                        mw = min(ms + 128, NP) - ms
                        if half == 0:
                            nc.scalar.copy(sup[b][0:mw, nt, :],
                                           sps[0:mw, 0:H])
                        else:
                            nc.vector.tensor_copy(sup[b][0:mw, nt, :],
                                                  sps[0:mw, H:2 * H])

            for b in (0, 1):
                sup[b] = supp.tile([128, NT, H], BF16, tag="sup",
                                   name="sup")
                # apply gn2 half 0 -> sup pairs 0-2, half 1 -> pairs 3-6
                for hi, (h0, hw) in enumerate(HALVES):
                    for ct in range(2):
                        apply_ra(ENG_GN2[ct][hi],
                                 y1[b][ct][:, h0:h0 + hw],
                                 y1[b][ct][:, h0:h0 + hw],
                                 ab2[b][ct][:, 0:1], ab2[b][ct][:, 1:2])
                    sup_pairs(b, range(0, 3) if hi == 0 else range(3, NPAIR))

            # --- agg (adjacency matmul) ---
            _mark("agg")
            y2 = {}
            y2stats = {}
            for b in (0, 1):
                y2[b] = [yp.tile([128, NP], F32R, tag="y1", name="y2")
                         for _ in range(2)]
                y2stats[b] = stats_new(2)
                for dt in range(2):
                    aps = [ps.tile([128, 512], F32, tag="ps", name="aps")
                           for _ in range(4)]
                    for kt in range(NT):
                        kn = min(128, N - kt * 128)
                        for ci, (f0, fw) in enumerate(FCH):
                            nc.tensor.matmul(
                                aps[ci][:, :fw],
                                sup[b][0:kn, kt,
                                       dt * 128:(dt + 1) * 128],
                                asb[0:kn, kt, f0:f0 + fw],
                                start=(kt == 0), stop=(kt == NT - 1))
                    for ci, (f0, fw) in enumerate(FCH):
                        evac("a",
                             y2[b][dt][:, f0:f0 + fw],
                             aps[ci][:, :fw], P(pn("conv_b"), dt))
                        note(y2stats[b][dt], ci, y2[b][dt], f0, fw)
            _mark("gn3")
            mr3 = {b: [gn_f1a(y2stats[b][dt]) for dt in range(2)]
                   for b in (0, 1)}

            # --- gn3 apply + lin2 + residual ---
            _mark("lin2")
            collect = pn_next is not None
            xstats = {}
            mrx = {0: {}, 1: {}}
            for b in (0, 1):
                for dt in range(2):
                    ab = gn_f1b(mr3[b][dt], pn("n2_g"), dt, pn("n2_b"))
                    for hi, (h0, hw) in enumerate(HALVES):
                        apply_ra(ENG_GN3[dt][hi], y2[b][dt][:, h0:h0 + hw],
                                 y2[b][dt][:, h0:h0 + hw],
                                 ab[:, 0:1], ab[:, 1:2])
                xstats[b] = stats_new(4) if collect else None
                lps = {}

                def ident_mm(mt, ci):
                    f0, fw = FCH[ci]
                    lps[(mt, ci)] = ps.tile([128, 512], F32, tag="ps",
                                            name="lps")
                    nc.tensor.matmul(lps[(mt, ci)][:, :fw], idm[:],
                                     xsrc(b, mt)[:, f0:f0 + fw],
                                     start=True, stop=False)

                ident_mm(0, 0)
                ident_mm(0, 1)
                for mt in range(4):
                    for ci, (f0, fw) in enumerate(FCH):
                        # keep the ident pipeline two chunks ahead
                        if ci < 2:
                            ident_mm(mt, ci + 2)
                        elif mt < 3:
                            ident_mm(mt + 1, ci - 2)
                        for ct in range(2):
                            nc.tensor.matmul(lps[(mt, ci)][:, :fw],
                                             l2w(ct, mt),
                                             y2[b][ct][:, f0:f0 + fw],
                                             start=False,
                                             stop=(ct == 1))
                        evac("a",
                             xdst[b][mt][:, f0:f0 + fw],
                             lps.pop((mt, ci))[:, :fw], l2bias(b, mt))
                        if collect:
                            note(xstats[b][mt], ci, xdst[b][mt], f0, fw)
                    if collect and mt >= 1:
                        mrx[b][mt - 1] = gn_f1a(xstats[b][mt - 1])
                if post is not None:
                    post(b)
            if not collect:
                return None
            mrx[0][3] = gn_f1a(xstats[0][3])
            mrx[1][3] = gn_f1a(xstats[1][3])
            xab = {}
            for b in (0, 1):
                xab[b] = [gn_f1b(mrx[b][mt], pn_next("pre_g"), mt,
                                 pn_next("pre_b"), abtag="abx", abbufs=8)
                          for mt in range(4)]
            return xab

        # ================= program =================
        rep = tc.For_i(0, nreps, 1) if nreps > 1 else contextlib.nullcontext()
        with rep:
            _mark("setup")
            # U per-channel stats (batch-independent)
            uch = cons.tile([128, 8, 2], F32, name="uch")
            for kt in range(8):
                ust = sm.tile([128, 4, 6], F32, tag="stats", bufs=14,
                              name="ust")
                for ci, (f0, fw) in enumerate(FCH):
                    ups = ps.tile([128, 512], F32, tag="ps", name="ups")
                    nc.tensor.matmul(ups[:, :fw],
                                     w0vt[:, kt * 128:(kt + 1) * 128],
                                     verts[:, f0:f0 + fw], start=True,
                                     stop=True)
                    rw = fw if f0 + fw <= N else (N - f0)
                    nc.vector.bn_stats(ust[:, ci, :], ups[:, 0:rw])
                nc.vector.bn_aggr(uch[:, kt, :], ust[:, :, :])
            # ---- b0 GN1: analytic coefficients per batch ----
            _mark("b0gn1")
            ab0 = {}
            for b in (0, 1):
                st3 = sm.tile([128, 8, 2], F32, tag="st3b", bufs=2,
                              name="st3b")
                sq = sm.tile([128, 8], F32, tag="sqb", bufs=2, name="sqb")
                nc.vector.tensor_tensor(st3[:, :, 0], uch[:, :, 0],
                                        vb[:, :, b], op=ALU.add)
                nc.vector.tensor_tensor(sq[:, :], st3[:, :, 0], st3[:, :, 0],
                                        op=ALU.mult)
                nc.vector.tensor_tensor(st3[:, :, 1], uch[:, :, 1], sq[:, :],
                                        op=ALU.add)
                ab = gn_chain8(st3, "b0_pre_g", "b0_pre_b",
                               abtag=f"ab0_{b}")
                t5 = sm.tile([128, 8], F32, tag="t5", bufs=2, name="t5")
                nc.vector.tensor_tensor(t5[:, :], ab[:, :, 0],
                                        vb[:, :, b], op=ALU.mult)
                nc.vector.tensor_tensor(ab[:, :, 1], ab[:, :, 1],
                                        t5[:, :], op=ALU.add)
                ab0[b] = ab

            # SU = (skW @ W0v) @ verts — emitted after the b0gn1 chains:
            # its PE matmuls cover the chains' DVE latency
            su = cons.tile([128, 4, NP], BF16, name="su")
            for ci, (f0, fw) in enumerate(FCH):
                for mt in range(4):
                    sps = ps.tile([128, 512], F32, tag="ps", name="spsu")
                    nc.tensor.matmul(sps[:, :fw],
                                     swt[:, mt * 128:(mt + 1) * 128],
                                     verts[:, f0:f0 + fw], start=True,
                                     stop=True)
                    nc.scalar.copy(su[:, mt, f0:f0 + fw], sps[:, :fw])

            # ---- b0 front: joint over batches (shared U chunks) ----
            _mark("b0front")
            y1 = {b: [yp.tile([128, NP], BF16, tag="y1", name="y1f")
                      for _ in range(2)] for b in (0, 1)}
            ystats = {b: stats_new(2) for b in (0, 1)}
            for ci, (f0, fw) in enumerate(FCH):
                y1ps = {b: [ps.tile([128, 512], F32, tag="ps", name="y1psf")
                            for _ in range(2)] for b in (0, 1)}

                def u_mm(kt):
                    ups = ps.tile([128, 512], F32, tag="ps", name="ups2")
                    nc.tensor.matmul(ups[:, :fw],
                                     w0vt[:, kt * 128:(kt + 1) * 128],
                                     verts[:, f0:f0 + fw],
                                     start=True, stop=True)
                    return ups

                ups = u_mm(0)   # one kt ahead so applies hide under PE work
                for kt in range(8):
                    x0r = {}
                    for b in (0, 1):
                        x0r[b] = xrp.tile([128, 432], BF16, tag="xr",
                                          name="x0r")
                        apply_ra(ENG_X0R[b], x0r[b][:, :fw], ups[:, :fw],
                                 ab0[b][:, kt, 0:1], ab0[b][:, kt, 1:2])
                    if kt < 7:
                        ups = u_mm(kt + 1)
                    for b in (0, 1):
                        for mt in range(2):
                            nc.tensor.matmul(
                                y1ps[b][mt][:, :fw],
                                b0l1[:, kt, mt * 128:(mt + 1) * 128],
                                x0r[b][:, :fw],
                                start=(kt == 0), stop=(kt == 7))
                for b in (0, 1):
                    for mt in range(2):
                        nc.scalar.activation(y1[b][mt][:, f0:f0 + fw],
                                             y1ps[b][mt][:, :fw], AF.Identity,
                                             bias=P("b0_lin1_b", mt))
                        note(ystats[b][mt], ci, y1[b][mt], f0, fw)

            if dump == 2:
                for b in (0, 1):
                    for mt in range(2):
                        nc.sync.dma_start(dbg_d.ap()[b * 4 + mt],
                                          y1[b][mt][:, :])

            # ---- b0 tail ----
            x = {b: [xp.tile([128, NP], X_DT, tag="x", name="x")
                     for _ in range(4)] for b in (0, 1)}
            xab = tail_pair(
                lambda s: "b0_" + s,
                lambda ct: cw0[:, ct, :],
                lambda ct, mt: l2t0[:, ct, mt * 128:(mt + 1) * 128],
                lambda b, mt: su[:, mt, :],
                x,
                lambda b, mt: svb2[:, mt, b:b + 1],
                y1, ystats, pn_next=lambda s: f"blk_{s}0", idm=identb)

            if dump == 1:
                for b in (0, 1):
                    for mt in range(4):
                        nc.sync.dma_start(dbg_d.ap()[b * 4 + mt],
                                          x[b][mt][:, :])

            # ---- head: stage 1 (h1/h2+stats) rides each batch's lin2;
            # stage 2 (chain+apply+h3+out) for both batches at the end ----
            hd = {}

            def emit_head(b):
                _mark("head")
                yh1 = yp.tile([64, NP], F32R, tag="y1", name="yh1")
                for (f0, fw) in FCH:
                    hps = ps.tile([64, 512], F32, tag="ps", name="hps")
                    for kt in range(4):
                        nc.tensor.matmul(hps[:, :fw], h1w[:, kt, :],
                                         x[b][kt][:, f0:f0 + fw],
                                         start=(kt == 0), stop=(kt == 3))
                    nc.scalar.activation(yh1[:, f0:f0 + fw], hps[:, :fw],
                                         AF.Relu, bias=P("h1_b", 0, 64))
                yh2 = yp.tile([32, NP], F32R, tag="y1", name="yh2")
                hstats = stats_new(1)
                for ci, (f0, fw) in enumerate(FCH):
                    hps2 = ps.tile([32, 512], F32, tag="ps", name="hps2")
                    nc.tensor.matmul(hps2[:, :fw], h2w[:], yh1[:, f0:f0 + fw],
                                     start=True, stop=True)
                    nc.scalar.activation(yh2[:, f0:f0 + fw], hps2[:, :fw],
                                         AF.Identity, bias=P("h2_b", 0, 32))
                    note(hstats[0], ci, yh2, f0, fw, parts=32)
                hd[b] = (yh2, hstats)

            def emit_head2(b):
                _mark("head")
                yh2, hstats = hd[b]
                mr = gn_f1a(hstats[0], parts=32, G=4)
                abh = gn_f1b(mr, "hn_g", 0, "hn_b", parts=32, G=4)
                for ci, (f0, fw) in enumerate(FCH):
                    apply_ra("v" if ci % 2 else "a", yh2[:, f0:f0 + fw],
                             yh2[:, f0:f0 + fw],
                             abh[0:32, 0:1], abh[0:32, 1:2])
                osb = wp.tile([4, NP], F32, tag="osb", bufs=2, name="osb")
                for (f0, fw) in FCH:
                    hps3 = ps.tile([4, 512], F32, tag="ps", name="hps3")
                    nc.tensor.matmul(hps3[:, :fw], h3w[:],
                                     yh2[:, f0:f0 + fw],
                                     start=True, stop=True)
                    nc.scalar.activation(osb[0:3, f0:f0 + fw],
                                         hps3[0:3, :fw],
                                         AF.Identity, bias=P("h3_b", 0, 3))
                nc.sync.dma_start(out_d.ap()[b], osb[0:3, 0:N])

            # ---- 5 residual blocks ----
            bw = {}

            def load_blk(i):
                bl1 = wp.tile([128, 4, H], BF16, tag="bl1", name="bl1")
                for ct in range(4):
                    nc.sync.dma_start(bl1[:, ct, :], d["bl1t"].ap()[i, ct])
                bcw = wp.tile([128, 2, H], BF16, tag="bcw", name="bcw")
                for ct in range(2):
                    nc.sync.dma_start(bcw[:, ct, :], d["bcw"].ap()[i, ct])
                bl2 = wp.tile([128, 2, C], F32R, tag="bl2", name="bl2")
                for ct in range(2):
                    nc.sync.dma_start(bl2[:, ct, :], d["bl2t"].ap()[i, ct])
                bw[i] = (bl1, bcw, bl2)

            load_blk(0)
            for i in range(L):
                bl1, bcw, bl2 = bw.pop(i)
                _mark("lin1")
                y1, ystats = lin1_pair(
                    lambda s, i=i: f"blk_{s}{i}", x, xab,
                    lambda ct, _w=bl1: _w[:, ct, :])
                if dump == 3 and i == 0:
                    for b in (0, 1):
                        for mt in range(2):
                            nc.sync.dma_start(dbg_d.ap()[b * 4 + mt],
                                              y1[b][mt][:, :])
                if i + 1 < L:
                    load_blk(i + 1)
                pn_next = (lambda s, j=i + 1: f"blk_{s}{j}") \
                    if i < L - 1 else None
                xab = tail_pair(
                    lambda s, i=i: f"blk_{s}{i}",
                    lambda ct, _w=bcw: _w[:, ct, :],
                    lambda ct, mt, _w=bl2: _w[:, ct, mt * 128:(mt + 1) * 128],
                    lambda b, mt: x[b][mt][:, :],
                    x,
                    lambda b, mt, i=i: P(f"blk_lin2_b{i}", mt),
                    y1, ystats, pn_next=pn_next, idm=identx)

            emit_head(0)
            emit_head(1)
            emit_head2(0)
            emit_head2(1)


    nc.compile()
    return nc


def _host_prep(inputs, fp8agg=True, xf32=True):
    f32 = np.float32
    bf = ml_dtypes.bfloat16
    shared = {}

    verts = np.zeros((4, NP), f32)
    verts[0:3, 0:N] = np.asarray(inputs["ref_vertices"], f32)
    shared["verts"] = verts

    src = np.asarray(inputs["adj_src"]).astype(np.int64)
    dst = np.asarray(inputs["adj_dst"]).astype(np.int64)
    w = np.asarray(inputs["adj_w"], f32)
    at = np.zeros((NT * 128, NP), f32)
    np.add.at(at, (src, dst), w)
    adt = ml_dtypes.float8_e4m3fn if fp8agg else bf
    shared["at"] = at.reshape(NT, 128, NP).astype(adt)

    lin0_W = np.asarray(inputs["lin0_W"], f32)
    skW = np.asarray(inputs["b0_skip_W"], f32)
    w0vt = np.zeros((4, 1024), f32)
    w0vt[0:3] = lin0_W[:, :3].T
    shared["w0vt"] = w0vt
    swt = np.zeros((4, 512), f32)
    swt[0:3] = (skW @ lin0_W[:, :3]).T
    shared["swt"] = swt

    ind = np.zeros((128, 16), f32)
    for c in range(128):
        ind[c, c // 8] = 1.0
    shared["g8"] = ind / 8.0
    shared["g8t"] = np.ascontiguousarray(ind.T)
    xdt = f32 if xf32 else bf
    shared["identb"] = np.eye(128).astype(bf)
    if xf32:
        shared["identr"] = np.eye(128, dtype=f32)

    sklin2_b = (skW @ np.asarray(inputs["lin0_b"], f32)
                + np.asarray(inputs["b0_skip_b"], f32)
                + np.asarray(inputs["b0_lin2_b"], f32))
    vals = {"lin0_b": inputs["lin0_b"],
            "b0_pre_g": inputs["b0_pre_g"], "b0_pre_b": inputs["b0_pre_b"],
            "b0_lin1_b": inputs["b0_lin1_b"],
            "b0_n1_g": inputs["b0_n1_g"], "b0_n1_b": inputs["b0_n1_b"],
            "b0_conv_b": inputs["b0_conv_b"],
            "b0_n2_g": inputs["b0_n2_g"], "b0_n2_b": inputs["b0_n2_b"],
            "b0_sklin2_b": sklin2_b,
            "h1_b": inputs["h1_b"], "h2_b": inputs["h2_b"],
            "hn_g": inputs["hn_g"], "hn_b": inputs["hn_b"],
            "h3_b": inputs["h3_b"]}
    for i in range(L):
        for nm, key in (("pre_g", "blk_pre_g"), ("pre_b", "blk_pre_b"),
                        ("lin1_b", "blk_lin1_b"), ("n1_g", "blk_n1_g"),
                        ("n1_b", "blk_n1_b"), ("conv_b", "blk_conv_b"),
                        ("n2_g", "blk_n2_g"), ("n2_b", "blk_n2_b"),
                        ("lin2_b", "blk_lin2_b")):
            vals[f"blk_{nm}{i}"] = np.asarray(inputs[key])[i]
    prm = np.zeros((128, NSLOT), f32)
    for (name, t), pos in PIDX.items():
        vec = np.asarray(vals[name], f32).ravel()
        seg = vec[t * 128:(t + 1) * 128]
        prm[0:len(seg), pos] = seg
    shared["prm"] = prm

    shared["b0l1t"] = np.ascontiguousarray(
        np.asarray(inputs["b0_lin1_W"], f32).T).reshape(8, 128, H).astype(bf)
    shared["b0cw"] = np.ascontiguousarray(
        np.asarray(inputs["b0_conv_W"], f32)).reshape(2, 128, H).astype(bf)
    shared["b0l2t"] = np.ascontiguousarray(
        np.asarray(inputs["b0_lin2_W"], f32).T).reshape(2, 128, C)
    shared["bl1t"] = np.ascontiguousarray(
        np.asarray(inputs["blk_lin1_W"], f32).transpose(0, 2, 1)).reshape(
            L, 4, 128, H).astype(bf)
    shared["bcw"] = np.ascontiguousarray(
        np.asarray(inputs["blk_conv_W"], f32)).reshape(L, 2, 128, H).astype(bf)
    shared["bl2t"] = np.ascontiguousarray(
        np.asarray(inputs["blk_lin2_W"], f32).transpose(0, 2, 1)).reshape(
            L, 2, 128, C)
    shared["h1t"] = np.ascontiguousarray(
        np.asarray(inputs["h1_W"], f32).T).reshape(4, 128, 64).astype(xdt)
    shared["h2t"] = np.ascontiguousarray(np.asarray(inputs["h2_W"], f32).T)
    h3t = np.zeros((32, 4), f32)
    h3t[:, 0:3] = np.asarray(inputs["h3_W"], f32).T
    shared["h3t"] = h3t

    img = np.asarray(inputs["image_resnet"], f32)
    lin0_b = np.asarray(inputs["lin0_b"], f32)
    vb_all = lin0_W[:, 3:] @ img.T + lin0_b[:, None]       # (1024, B)
    svb_all = skW @ (lin0_W[:, 3:] @ img.T) + sklin2_b[:, None]  # (512, B)
    in_maps = []
    for c in range(NCORES):
        m = dict(shared)
        vb_c = vb_all[:, c * BLOC:(c + 1) * BLOC]
        m["vbh"] = np.ascontiguousarray(
            vb_c.reshape(8, 128, BLOC).transpose(1, 0, 2))
        svb_c = svb_all[:, c * BLOC:(c + 1) * BLOC]
        m["svbh"] = np.ascontiguousarray(
            svb_c.reshape(4, 128, BLOC).transpose(1, 0, 2))
        in_maps.append(m)
    return in_maps


_NC_CACHE = {}


def _get_nc(nreps=1, **kw):
    key = (nreps, tuple(sorted(kw.items())))
    if key not in _NC_CACHE:
        _NC_CACHE[key] = build(nreps, **kw)
    return _NC_CACHE[key]


def run_on_hw(inputs, nreps=1, **kw):
    nc = _get_nc(nreps, **kw)
    in_maps = _host_prep(inputs, fp8agg=kw.get("fp8agg", True),
                         xf32=kw.get("xf32", True))
    res = run_bass_kernel_spmd(nc, in_maps, core_ids=list(range(NCORES)),
                               trace=False)
    return np.concatenate([res.results[c]["out"] for c in range(NCORES)],
                          axis=0)


def run_dbg(inputs, dump, **kw):
    nc = _get_nc(1, dump=dump, **kw)
    in_maps = _host_prep(inputs, fp8agg=kw.get("fp8agg", True),
                         xf32=kw.get("xf32", True))
    res = run_bass_kernel_spmd(nc, in_maps, core_ids=list(range(NCORES)),
                               trace=False)
    return res.results[0]["dbg"]


def kernel(**inputs) -> np.ndarray:
    return run_on_hw(inputs, nreps=1)
